# revision 1
# baseline (speedup 1.0000x reference)
"""Trainium2 Bass kernel for nn_AttentiveSSM (sparse chunked attention + SSM).

Sharding (8 cores, tensor-parallel over heads):
  core c owns q-heads {2c, 2c+1} and kv-head c//2. Each core computes its
  Q/K/V projections from the full (transposed) x, runs the chunked SSM +
  RoPE, sparse attention against the compressed key set (chunk boundaries +
  first-4 + t-1 diagonal), and a partial output projection through its wo
  column slice. Host sums the 8 partial yT outputs.

Self-contained: hardcodes all shapes; no sibling imports.
"""
import sys
import numpy as np

sys.path.insert(0, '/opt/trn_rl_repo')

import concourse.bacc as bacc               # noqa: E402
import concourse.mybir as mybir             # noqa: E402
from concourse.tile import TileContext      # noqa: E402
from concourse import bass_utils            # noqa: E402
from concourse.alu_op_type import AluOpType # noqa: E402

# silence cloud artifact upload in traced runs
bass_utils.upload_artifacts = lambda tmpdir: tmpdir

S = 2048          # sequence
D = 2048          # model dim
HD = 128          # head dim
QB = 512          # query block
NSB = S // QB     # 4 s-blocks
NKT = D // 128    # 16 contraction tiles
KC = 8            # token chunk
NEG = -1.0e9
SCALE = float(1.0 / np.sqrt(HD))

F32 = mybir.dt.float32
F32R = mybir.dt.float32r
MUL = AluOpType.mult
ADD = AluOpType.add
SUB = AluOpType.subtract

_CACHE = {}


def _build_module():
    nc = bacc.Bacc("TRN2", num_devices=8)

    def din(name, shape, dt=F32R):
        return nc.dram_tensor(name, list(shape), dt, kind="ExternalInput")

    xT = din("xT", (D, S))
    wqT = din("wqT", (D, 256))
    wkT = din("wkT", (D, 128))
    wvT = din("wvT", (D, 128))
    woT0 = din("woT0", (128, D))
    woT1 = din("woT1", (128, D))
    cosk = din("cosk", (128, S), F32)   # halves duplicated (base-partition rule)
    sink = din("sink", (128, S), F32)
    akp = din("akp", (128, QB), F32)   # scan decay pattern (0 at i%8==0)
    avp = din("avp", (128, QB), F32)
    bk = din("bk", (128, 1), F32)
    ck = din("ck", (128, 1), F32)
    bv = din("bv", (128, 1), F32)
    cv = din("cv", (128, 1), F32)
    bandm = din("bandm", (64, QB), F32)
    f4m = din("f4m", (4, QB), F32)
    mdiag = din("mdiag", (4, QB), F32)
    ident = din("ident", (128, 128))
    onesrowf = din("onesrowf", (1, 128), F32)
    ones = din("ones", (128, 1))
    onesrow = din("onesrow", (1, 128))
    zeros = din("zeros", (128, 1))
    yT = nc.dram_tensor("yT", [D, S], F32, kind="ExternalOutput")

    with TileContext(nc) as tc:
        with (
            tc.tile_pool(name="const", bufs=1) as cp,
            tc.tile_pool(name="big", bufs=1) as bp,
            tc.tile_pool(name="xs", bufs=10) as xs,
            tc.tile_pool(name="tmp", bufs=2) as tp,
            tc.tile_pool(name="pp", bufs=4) as ppool,
            tc.tile_pool(name="at", bufs=1, space="PSUM") as at,
        ):
            # ---- constants ----
            def cload(name, shape, src, dt=F32):
                t = cp.tile(list(shape), dt, tag=name, name=name)
                nc.gpsimd.dma_start(t[:], src[:])
                return t

            wq_sb = cp.tile([128, NKT * 256], F32R, tag="wq")
            wk_sb = cp.tile([128, NKT * 128], F32R, tag="wk")
            wv_sb = cp.tile([128, NKT * 128], F32R, tag="wv")
            for k in range(NKT):
                nc.gpsimd.dma_start(wq_sb[:, k * 256:(k + 1) * 256],
                                    wqT[128 * k:128 * (k + 1), :])
                nc.gpsimd.dma_start(wk_sb[:, k * 128:(k + 1) * 128],
                                    wkT[128 * k:128 * (k + 1), :])
                nc.gpsimd.dma_start(wv_sb[:, k * 128:(k + 1) * 128],
                                    wvT[128 * k:128 * (k + 1), :])
            cosk_s = cload("cosk", (128, S), cosk)
            sink_s = cload("sink", (128, S), sink)
            akp_s = cload("akp", (128, QB), akp)
            avp_s = cload("avp", (128, QB), avp)
            bk_s = cload("bk", (128, 1), bk)
            ck_s = cload("ck", (128, 1), ck)
            bv_s = cload("bv", (128, 1), bv)
            cv_s = cload("cv", (128, 1), cv)
            bandm_s = cload("bandm", (64, QB), bandm)
            f4m_s = cload("f4m", (4, QB), f4m)
            mdiag_s = [cload(f"mdiag{b}", (1, QB), mdiag[b:b + 1, :])
                       for b in range(4)]
            ident_s = cload("ident", (128, 128), ident, F32R)
            ones_s = cload("ones", (128, 1), ones, F32R)
            onesrow_s = cload("onesrow", (1, 128), onesrow, F32R)
            zeros_s = cload("zeros", (128, 1), zeros, F32R)
            onesrowf_s = cload("onesrowf", (1, 128), onesrowf, F32)
            wo_sb0 = cload("wo0", (128, D), woT0, F32R)
            wo_sb1 = cload("wo1", (128, D), woT1, F32R)

            # ---- big state ----
            QT0 = bp.tile([128, S], F32R, tag="QT0")
            QT1 = bp.tile([128, S], F32R, tag="QT1")
            KTp = bp.tile([128, S], F32R, tag="KTp")
            VTp = bp.tile([128, S], F32R, tag="VTp")
            KCt = bp.tile([128, 264], F32R, tag="KCt")   # compressed keys
            VG = bp.tile([128, 264], F32R, tag="VG")     # gathered V (hd-major)
            vc = [bp.tile([64, 128], F32R, tag=f"vc{t}", name=f"vc{t}")
                  for t in range(4)]
            vc4 = bp.tile([4, 128], F32R, tag="vc4")
            OT0 = bp.tile([128, S], F32R, tag="OT0")
            OT1 = bp.tile([128, S], F32R, tag="OT1")

            def proj_sblock(sb, pj):
                s0 = QB * sb

                def rope(dst, dst_lo, src, cos_t, sin_t):
                    # dst = src*cos2 + swap(src)*sin2  (sin2 = [sin; -sin])
                    t = tp.tile([128, QB], F32, tag="ropet", bufs=1, name="ropet")
                    u = tp.tile([128, QB], F32, tag="ropeu", bufs=1, name="ropeu")
                    nc.vector.tensor_tensor(t[:], src[:], cos_t[:, s0:s0 + QB], MUL)
                    nc.vector.tensor_tensor(u[0:64, :], src[64:128, :],
                                            sin_t[64:128, s0:s0 + QB], MUL)
                    nc.vector.tensor_tensor(u[64:128, :], src[0:64, :],
                                            sin_t[0:64, s0:s0 + QB], MUL)
                    nc.vector.tensor_tensor(dst[dst_lo:dst_lo + 128, s0:s0 + QB],
                                            t[:], u[:], ADD)

                psq0 = pj.tile([128, QB], F32, tag="q0")
                psq1 = pj.tile([128, QB], F32, tag="q1")
                psk = pj.tile([128, QB], F32, tag="k")
                psv = pj.tile([128, QB], F32, tag="v")
                for half in range(2):
                    xts = []
                    for kk in range(8):
                        k = half * 8 + kk
                        xt = xs.tile([128, QB], F32R, tag="x", name=f"x{sb}_{k}")
                        nc.sync.dma_start(xt[:],
                                          xT[128 * k:128 * (k + 1), s0:s0 + QB])
                        xts.append(xt)
                    for kk in range(8):
                        k = half * 8 + kk
                        nc.tensor.matmul(psq0[:], wq_sb[:, k * 256:k * 256 + 128],
                                         xts[kk][:], start=(k == 0),
                                         stop=(k == NKT - 1))
                    for kk in range(8):
                        k = half * 8 + kk
                        nc.tensor.matmul(psq1[:],
                                         wq_sb[:, k * 256 + 128:k * 256 + 256],
                                         xts[kk][:], start=(k == 0),
                                         stop=(k == NKT - 1))
                    for kk in range(8):
                        k = half * 8 + kk
                        nc.tensor.matmul(psk[:], wk_sb[:, k * 128:(k + 1) * 128],
                                         xts[kk][:], start=(k == 0),
                                         stop=(k == NKT - 1))
                    for kk in range(8):
                        k = half * 8 + kk
                        nc.tensor.matmul(psv[:], wv_sb[:, k * 128:(k + 1) * 128],
                                         xts[kk][:], start=(k == 0),
                                         stop=(k == NKT - 1))
                # rope Q0 first (frees q0 psum bank earliest)
                rope(QT0, 0, psq0, cosk_s, sink_s)
                # K chain: SSM -> rope -> gather (feeds next scores soonest)
                bkt = tp.tile([128, QB], F32, tag="bkt", bufs=1)
                nc.vector.tensor_scalar_mul(bkt[:], psk[:], bk_s[:])
                hk = tp.tile([128, QB], F32, tag="hk", bufs=1)
                nc.vector.tensor_tensor_scan(hk[:], akp_s[:], bkt[:], 0.0, MUL, ADD)
                kp = tp.tile([128, QB], F32, tag="kp", bufs=1)
                nc.vector.scalar_tensor_tensor(kp[:], hk[:], ck_s[:], psk[:], MUL, ADD)
                rope(KTp, 0, kp, cosk_s, sink_s)
                nc.gpsimd.tensor_copy(KCt[:, 64 * sb:64 * (sb + 1)],
                                      KTp[:, s0 + 7:s0 + QB:8])
                if sb == 0:
                    nc.gpsimd.tensor_copy(KCt[:, 256:260], KTp[:, 0:4])
                # rope Q1
                rope(QT1, 0, psq1, cosk_s, sink_s)
                # V chain: SSM -> gather
                bvt = tp.tile([128, QB], F32, tag="bvt", bufs=1)
                nc.vector.tensor_scalar_mul(bvt[:], psv[:], bv_s[:])
                hv = tp.tile([128, QB], F32, tag="hv", bufs=1)
                nc.vector.tensor_tensor_scan(hv[:], avp_s[:], bvt[:], 0.0, MUL, ADD)
                nc.vector.scalar_tensor_tensor(VTp[:, s0:s0 + QB], hv[:], cv_s[:],
                                               psv[:], MUL, ADD)
                nc.gpsimd.tensor_copy(VG[:, 64 * sb:64 * (sb + 1)],
                                      VTp[:, s0 + 7:s0 + QB:8])
                if sb == 0:
                    nc.gpsimd.tensor_copy(VG[:, 256:260], VTp[:, 0:4])
                # transpose V chunk -> vc[sb] (64,128)
                pst = at.tile([64, 128], F32R, tag="sc")
                nc.tensor.transpose(pst[:], VG[:, 64 * sb:64 * (sb + 1)], ident_s[:])
                nc.vector.tensor_copy(vc[sb][:], pst[:])
                if sb == 0:
                    pst4 = at.tile([4, 128], F32R, tag="sc")
                    nc.tensor.transpose(pst4[:], VG[:, 256:260], ident_s[:])
                    nc.vector.tensor_copy(vc4[:], pst4[:])

            def attn_scores(b, h):
                q0 = QB * b
                QTh = QT0 if h == 0 else QT1
                plist = []   # (P tile, rows, vc tile)
                for t in range(b + 1):
                    st = at.tile([64, QB], F32, tag="sc", name=f"st{b}_{h}_{t}")
                    nc.tensor.matmul(st[:], KCt[:, 64 * t:64 * (t + 1)],
                                     QTh[:, q0:q0 + QB], start=True, stop=True)
                    if t == b:
                        nc.vector.tensor_tensor(st[:], st[:], bandm_s[:], ADD)
                    P = tp.tile([64, QB], F32R, tag="P", bufs=8,
                                name=f"P{b}_{h}_{t}")
                    nc.scalar.activation(P[:], st[:],
                                         mybir.ActivationFunctionType.Exp,
                                         scale=SCALE)
                    plist.append((P, 64, vc[t]))
                st4 = at.tile([4, QB], F32, tag="sc", name=f"st4_{b}_{h}")
                nc.tensor.matmul(st4[:], KCt[:, 256:260], QTh[:, q0:q0 + QB],
                                 start=True, stop=True)
                if b == 0:
                    nc.vector.tensor_tensor(st4[:], st4[:], f4m_s[:], ADD)
                P4 = tp.tile([4, QB], F32R, tag="P4", bufs=2, name=f"P4_{b}_{h}")
                nc.scalar.activation(P4[:], st4[:],
                                     mybir.ActivationFunctionType.Exp,
                                     scale=SCALE)
                plist.append((P4, 4, vc4))

                # diagonal (t-1) term
                z = tp.tile([128, QB], F32R, tag="z", bufs=1, name=f"z{b}_{h}")
                if b == 0:
                    nc.vector.tensor_copy(z[:, 0:1], zeros_s[:])
                    nc.vector.tensor_tensor(z[:, 1:QB], QTh[:, 1:QB].bitcast(F32),
                                            KTp[:, 0:QB - 1].bitcast(F32), MUL)
                else:
                    nc.vector.tensor_tensor(z[:], QTh[:, q0:q0 + QB].bitcast(F32),
                                            KTp[:, q0 - 1:q0 + QB - 1].bitcast(F32),
                                            MUL)
                sd = at.tile([1, QB], F32, tag="sc", name=f"sd{b}_{h}")
                nc.tensor.matmul(sd[:], ones_s[:], z[:], start=True, stop=True)
                sdm = tp.tile([1, QB], F32, tag="sdm", bufs=1, name=f"sdm{b}_{h}")
                nc.vector.tensor_tensor(sdm[:], sd[:], mdiag_s[b][:], ADD)
                pd = tp.tile([1, QB], F32R, tag="pd", bufs=2, name=f"pd{b}_{h}")
                nc.scalar.activation(pd[:], sdm[:],
                                     mybir.ActivationFunctionType.Exp,
                                     scale=SCALE)
                return plist, pd

            def attn_tail(b, h, plist, pd):
                q0 = QB * b
                OTh = OT0 if h == 0 else OT1
                den = at.tile([1, QB], F32, tag="den", name=f"den{b}_{h}")
                oun = at.tile([128, QB], F32, tag="oun", name=f"oun{b}_{h}")
                n = len(plist)
                for i, (P, rows, _) in enumerate(plist):
                    nc.tensor.matmul(den[:], ones_s[0:rows, :], P[:],
                                     start=(i == 0), stop=(i == n - 1))
                for i, (P, rows, vt) in enumerate(plist):
                    nc.tensor.matmul(oun[:], vt[:], P[:],
                                     start=(i == 0), stop=(i == n - 1))
                nc.vector.tensor_tensor(den[:], den[:], pd[:].bitcast(F32), ADD)

                psb = at.tile([128, QB], F32, tag="sc", name=f"psb{b}_{h}")
                nc.tensor.matmul(psb[:], onesrow_s[:], pd[:], start=True, stop=True)
                vsh = tp.tile([128, QB], F32, tag="vsh", bufs=1, name=f"vsh{b}_{h}")
                if b == 0:
                    nc.vector.tensor_copy(vsh[:, 0:1], zeros_s[:].bitcast(F32))
                    nc.vector.tensor_tensor(vsh[:, 1:QB], psb[:, 1:QB],
                                            VTp[:, 0:QB - 1].bitcast(F32), MUL)
                else:
                    nc.vector.tensor_tensor(vsh[:], psb[:],
                                            VTp[:, q0 - 1:q0 + QB - 1].bitcast(F32),
                                            MUL)
                nc.vector.tensor_tensor(oun[:], oun[:], vsh[:], ADD)

                # normalize -> OT
                rec_f = tp.tile([1, QB], F32, tag="recf", bufs=1, name=f"recf{b}_{h}")
                nc.vector.reciprocal_approx_fast(rec_f[:], den[:])
                rb = at.tile([128, QB], F32, tag="sc", name=f"rb{b}_{h}")
                nc.tensor.matmul(rb[:], onesrowf_s[:], rec_f[:], start=True,
                                 stop=True)
                rbs = tp.tile([128, QB], F32, tag="rbs", bufs=1, name=f"rbs{b}_{h}")
                nc.scalar.copy(rbs[:], rb[:])
                nc.vector.tensor_tensor(OTh[:, q0:q0 + QB], oun[:], rbs[:], MUL)

            def wo_block(sb, pj):
                s0 = QB * sb
                for dtile in range(NKT):
                    d0 = 128 * dtile
                    yp = pj.tile([128, QB], F32,
                                 tag=["q0", "q1", "k", "v"][dtile % 4],
                                 name=f"yp{sb}_{dtile}")
                    nc.tensor.matmul(yp[:], wo_sb0[:, d0:d0 + 128],
                                     OT0[:, s0:s0 + QB], start=True, stop=False)
                    nc.tensor.matmul(yp[:], wo_sb1[:, d0:d0 + 128],
                                     OT1[:, s0:s0 + QB], start=False, stop=True)
                    yt = tp.tile([128, QB], F32, tag="yt")
                    if dtile % 2 == 0:
                        nc.vector.tensor_copy(yt[:], yp[:])
                    else:
                        nc.scalar.copy(yt[:], yp[:])
                    nc.sync.dma_start(yT[d0:d0 + 128, s0:s0 + QB], yt[:])

            # ---- phase interleave: scores issued early, tails after dense
            # PE work so exp/DVE latency is absorbed ----
            with tc.tile_pool(name="pj", bufs=1, space="PSUM") as pj:
                proj_sblock(0, pj)
                proj_sblock(1, pj)
                a00 = attn_scores(0, 0)
                a01 = attn_scores(0, 1)
                proj_sblock(2, pj)
                attn_tail(0, 0, *a00)
                attn_tail(0, 1, *a01)
                a10 = attn_scores(1, 0)
                a11 = attn_scores(1, 1)
                proj_sblock(3, pj)
                wo_block(0, pj)
                attn_tail(1, 0, *a10)
                attn_tail(1, 1, *a11)
                a20 = attn_scores(2, 0)
                a21 = attn_scores(2, 1)
                wo_block(1, pj)
                attn_tail(2, 0, *a20)
                attn_tail(2, 1, *a21)
                a30 = attn_scores(3, 0)
                a31 = attn_scores(3, 1)
                wo_block(2, pj)
                attn_tail(3, 0, *a30)
                attn_tail(3, 1, *a31)
                wo_block(3, pj)

    nc.compile()
    return nc


def _softplus(x):
    return np.log1p(np.exp(-np.abs(x))) + np.maximum(x, 0)


def _host_prep(inputs):
    x = np.asarray(inputs['x'], np.float32)
    freq = np.asarray(inputs['freq_cis'], np.float32)
    wq = np.asarray(inputs['wq'], np.float32)
    wk = np.asarray(inputs['wk'], np.float32)
    wv = np.asarray(inputs['wv'], np.float32)
    wo = np.asarray(inputs['wo'], np.float32)
    alk = np.asarray(inputs['a_log_k'], np.float32)
    bk = np.asarray(inputs['b_k'], np.float32)
    ck = np.asarray(inputs['c_k'], np.float32)
    alv = np.asarray(inputs['a_log_v'], np.float32)
    bv = np.asarray(inputs['b_v'], np.float32)
    cv = np.asarray(inputs['c_v'], np.float32)

    perm = np.concatenate([np.arange(0, HD, 2), np.arange(1, HD, 2)])
    xT = np.ascontiguousarray(x[0].T)                      # (D, S)
    cos = np.ascontiguousarray(freq[:, :, 0, 0].T)         # (64, S)
    sin = np.ascontiguousarray(freq[:, :, 1, 0].T)

    idx = np.arange(QB)
    bandm = np.full((64, QB), NEG, np.float32)
    for r in range(64):
        bandm[r, 8 * r + 8:] = 0.0
    f4m = np.full((4, QB), NEG, np.float32)
    for k in range(4):
        f4m[k, k:] = 0.0
    t = np.arange(S)
    mdiag = np.where((t >= 5) & (t % 8 != 0), 0.0, NEG).astype(np.float32)
    mdiag = mdiag.reshape(4, QB)

    cos2 = np.concatenate([cos, cos], 0)
    sin2 = np.concatenate([sin, -sin], 0)
    shared = {
        "xT": xT,
        "cosk": cos2, "sink": sin2,
        "bandm": bandm, "f4m": f4m, "mdiag": mdiag,
        "ident": np.eye(128, dtype=np.float32),
        "ones": np.ones((128, 1), np.float32),
        "onesrow": np.ones((1, 128), np.float32),
        "onesrowf": np.ones((1, 128), np.float32),
        "zeros": np.zeros((128, 1), np.float32),
    }

    ak_full = np.exp(-_softplus(alk.astype(np.float64))).astype(np.float32)
    av_full = np.exp(-_softplus(alv.astype(np.float64))).astype(np.float32)
    col = np.arange(QB)

    in_maps = []
    for c in range(8):
        g = c // 2
        wq_c = wq[256 * c:256 * (c + 1)]
        wq_cp = np.concatenate([wq_c[h * HD:(h + 1) * HD][perm] for h in range(2)])
        wk_g = wk[128 * g:128 * (g + 1)][perm]
        wv_g = wv[128 * g:128 * (g + 1)]
        ak = ak_full[128 * g:128 * (g + 1)][perm]
        bk_g = bk[128 * g:128 * (g + 1)][perm]
        ck_g = ck[128 * g:128 * (g + 1)][perm]
        av_ = av_full[128 * g:128 * (g + 1)]
        bv_g = bv[128 * g:128 * (g + 1)]
        cv_g = cv[128 * g:128 * (g + 1)]
        akp = np.where(col[None, :] % KC == 0, 0.0, ak[:, None]).astype(np.float32)
        avp = np.where(col[None, :] % KC == 0, 0.0, av_[:, None]).astype(np.float32)
        m = dict(shared)
        m.update({
            "wqT": np.ascontiguousarray(wq_cp.T),
            "wkT": np.ascontiguousarray(wk_g.T),
            "wvT": np.ascontiguousarray(wv_g.T),
            "woT0": np.ascontiguousarray(wo[:, 256 * c:256 * c + 128].T),
            "woT1": np.ascontiguousarray(wo[:, 256 * c + 128:256 * (c + 1)].T),
            "akp": akp, "avp": avp,
            "bk": bk_g[:, None].astype(np.float32),
            "ck": ck_g[:, None].astype(np.float32),
            "bv": bv_g[:, None].astype(np.float32),
            "cv": cv_g[:, None].astype(np.float32),
        })
        in_maps.append(m)
    return in_maps


def kernel(**inputs) -> np.ndarray:
    if 'nc' not in _CACHE:
        _CACHE['nc'] = _build_module()
    nc = _CACHE['nc']
    in_maps = _host_prep(inputs)
    res = bass_utils.run_bass_kernel_spmd(nc, in_maps, core_ids=list(range(8)),
                                          **_CACHE.get('run_kwargs', {}))
    _CACHE['last_result'] = res
    yT = res.results[0]["yT"].astype(np.float64)
    for c in range(1, 8):
        yT += res.results[c]["yT"]
    return np.ascontiguousarray(yT.T[None]).astype(np.float32)



# revision 11
# speedup vs baseline: 1.6142x; 1.6142x over previous
"""Trainium2 Bass kernel for nn_AttentiveSSM (sparse chunked attention + SSM).

Sharding (8 cores, tensor-parallel over heads):
  core c owns q-heads {2c, 2c+1} and kv-head c//2. Each core computes its
  Q/K/V projections from the full (transposed) x in bf16, runs the chunked
  SSM + RoPE, sparse attention against the compressed key set (chunk
  boundaries + first-4 + t-1 diagonal), and a partial output projection
  through its wo column slice. Host sums the 8 partial yT outputs.

All PE operands are bf16 (1 cycle/row, fast weight load); accumulation is
fp32 in PSUM. Scores are grouped (G0=[first4|chunk0], C1=chunk1,
G1=[chunk1|chunk2], G2=chunk3) so each score/PV matmul streams a full
512-row block. K/V histories carry a leading zero column so the t-1
diagonal has no block-edge special case.

Self-contained: hardcodes all shapes; no sibling imports.
"""
import sys
import numpy as np

sys.path.insert(0, '/opt/trn_rl_repo')

import concourse.bacc as bacc               # noqa: E402
import concourse.mybir as mybir             # noqa: E402
from concourse.tile import TileContext      # noqa: E402
from concourse import bass_utils            # noqa: E402
from concourse.alu_op_type import AluOpType # noqa: E402

# silence cloud artifact upload in traced runs
bass_utils.upload_artifacts = lambda tmpdir: tmpdir

S = 2048          # sequence
D = 2048          # model dim
HD = 128          # head dim
QB = 512          # query block
NKT = D // 128    # 16 contraction tiles
KC = 8            # token chunk
NEG = -1.0e9
SCALE = float(1.0 / np.sqrt(HD))

F32 = mybir.dt.float32
F32R = mybir.dt.float32r
BF16 = mybir.dt.bfloat16
MUL = AluOpType.mult
ADD = AluOpType.add
EXP = mybir.ActivationFunctionType.Exp

# f32c column layout
MG0B0, MG1B2, MG2B3, AKP, AVP = 0, 512, 1024, 1536, 2048
CBK, CBV, ONESF = 2560, 2561, 2562
F32C_W = 2690
# bf16c column layout (mdiag lives in row 0 as four 512-col blocks)
MDIAG, IDENT, ONES = 0, 2048, 2176
BF16C_W = 2304

# key-group layout inside KCt/VG (free dim):
#   [0:4] first4, [4:68] chunk0 -> G0 [0:68]
#   [68:132] chunk1 (C1), [132:196] chunk2 -> G1 [68:196]
#   [196:260] chunk3 -> G2
G0_LO, G0_N = 0, 68
C1_LO, C1_N = 68, 64
G1_LO, G1_N = 68, 128
G2_LO, G2_N = 196, 64

_CACHE = {}


def _build_module():
    nc = bacc.Bacc("TRN2", num_devices=8)

    def din(name, shape, dt):
        return nc.dram_tensor(name, list(shape), dt, kind="ExternalInput")

    xT = din("xT", (D, S), BF16)
    wq = din("wq", (128, NKT * 256), BF16)
    wk = din("wk", (128, NKT * 128), BF16)
    wv = din("wv", (128, NKT * 128), BF16)
    wo = din("wo", (128, 2 * D), BF16)
    cosk = din("cosk", (128, S), F32)
    sink = din("sink", (128, S), F32)
    f32c = din("f32c", (128, F32C_W), F32)
    bf16c = din("bf16c", (128, BF16C_W), BF16)
    yT = nc.dram_tensor("yT", [D, S], BF16, kind="ExternalOutput")
    dbg = {}
    for nm, sh in [("dQT0", (128, S)), ("dKTx", (128, S + 1)),
                   ("dVTx", (128, S + 1)), ("dKCt", (128, 260)),
                   ("dVG", (128, 260)), ("dOT0", (128, S)),
                   ("dOT1", (128, S))]:
        dbg[nm] = nc.dram_tensor(nm, list(sh), BF16, kind="ExternalOutput")

    with TileContext(nc) as tc:
        with (
            tc.tile_pool(name="const", bufs=1) as cp,
            tc.tile_pool(name="xs", bufs=32) as xs,
            tc.tile_pool(name="big", bufs=1) as bp,
            tc.tile_pool(name="tp", bufs=2) as tp,
            tc.tile_pool(name="pj", bufs=1, space="PSUM") as pj,
            tc.tile_pool(name="at", bufs=1, space="PSUM") as at,
        ):
            # ---- constants (each one DMA) ----
            wq_sb = cp.tile([128, NKT * 256], BF16, tag="wq")
            nc.sync.dma_start(wq_sb[:], wq[:])
            wk_sb = cp.tile([128, NKT * 128], BF16, tag="wk")
            nc.sync.dma_start(wk_sb[:], wk[:])
            wv_sb = cp.tile([128, NKT * 128], BF16, tag="wv")
            nc.sync.dma_start(wv_sb[:], wv[:])
            cos_sb = cp.tile([128, S], F32, tag="cos")
            nc.sync.dma_start(cos_sb[:], cosk[:])
            sin_sb = cp.tile([128, S], F32, tag="sin")
            nc.sync.dma_start(sin_sb[:], sink[:])
            fc = cp.tile([128, F32C_W], F32, tag="f32c")
            nc.sync.dma_start(fc[:], f32c[:])
            bc = cp.tile([128, BF16C_W], BF16, tag="bf16c")
            nc.sync.dma_start(bc[:], bf16c[:])
            wo_sb = cp.tile([128, 2 * D], BF16, tag="wo")
            nc.sync.dma_start(wo_sb[:], wo[:])

            # ---- x tiles: per (sblock, ktile), bf16 [128, 512].
            # sb2/sb3 loads are emitted after proj 0/1 so their slot-reuse
            # waits cannot block the issuing queues. ----
            xt = {}

            def load_x(sb):
                for k in range(NKT):
                    t = xs.tile([128, QB], BF16, tag="x", name=f"x{sb}_{k}")
                    eng = nc.sync if k % 2 == 0 else nc.gpsimd
                    eng.dma_start(t[:], xT[128 * k:128 * (k + 1),
                                           QB * sb:QB * (sb + 1)])
                    xt[(sb, k)] = t

            load_x(0)
            load_x(1)

            # ---- big state ----
            QT0 = bp.tile([128, S], BF16, tag="QT0")
            QT1 = bp.tile([128, S], BF16, tag="QT1")
            KTx = bp.tile([128, S + 1], BF16, tag="KTx")   # col0 = 0 pad
            VTx = bp.tile([128, S + 1], BF16, tag="VTx")
            OT0 = bp.tile([128, S], BF16, tag="OT0")
            OT1 = bp.tile([128, S], BF16, tag="OT1")
            KCt = bp.tile([128, 260], BF16, tag="KCt")
            VG = bp.tile([128, 260], BF16, tag="VG")
            vcG0 = bp.tile([G0_N, 128], BF16, tag="vcG0")
            vcC1 = bp.tile([C1_N, 128], BF16, tag="vcC1")
            vcG1 = bp.tile([G1_N, 128], BF16, tag="vcG1")
            vcG2 = bp.tile([G2_N, 128], BF16, tag="vcG2")
            VCS = {'G0': vcG0, 'C1': vcC1, 'G1': vcG1, 'G2': vcG2}

            nc.vector.memset(KTx[:, 0:1], 0)
            nc.vector.memset(VTx[:, 0:1], 0)

            def rope(label, src, s0):
                # src must be SBUF (gpsimd cannot read PSUM).
                # t + u = src*cos2 + swap(src)*sin2 ; sin2 = [sin; -sin]
                t = tp.tile([128, QB], F32, tag="ropet", name=f"rt{label}")
                u = tp.tile([128, QB], F32, tag="ropeu", name=f"ru{label}")
                nc.vector.tensor_tensor(t[:], src[:], cos_sb[:, s0:s0 + QB], MUL)
                nc.gpsimd.tensor_tensor(u[0:64, :], src[64:128, :],
                                        sin_sb[64:128, s0:s0 + QB], MUL)
                nc.gpsimd.tensor_tensor(u[64:128, :], src[0:64, :],
                                        sin_sb[0:64, s0:s0 + QB], MUL)
                return t, u

            def proj_sblock(sb):
                s0 = QB * sb
                qq = pj.tile([128, 2 * QB], F32, tag="qq", name=f"qq_{sb}")
                kv = pj.tile([128, 2 * QB], F32, tag="kv", name=f"kv_{sb}")
                psq0 = qq[:, 0:QB]
                psq1 = qq[:, QB:2 * QB]
                psk = kv[:, 0:QB]
                psv = kv[:, QB:2 * QB]
                for half in range(2):
                    ks = [half * 8 + kk for kk in range(8)]
                    for k in ks:
                        nc.tensor.matmul(psq0, wq_sb[:, k * 256:k * 256 + 128],
                                         xt[(sb, k)][:], start=(k == 0),
                                         stop=(k == NKT - 1))
                    for k in ks:
                        nc.tensor.matmul(psq1,
                                         wq_sb[:, k * 256 + 128:k * 256 + 256],
                                         xt[(sb, k)][:], start=(k == 0),
                                         stop=(k == NKT - 1))
                    for k in ks:
                        nc.tensor.matmul(psk, wk_sb[:, k * 128:(k + 1) * 128],
                                         xt[(sb, k)][:], start=(k == 0),
                                         stop=(k == NKT - 1))
                    for k in ks:
                        nc.tensor.matmul(psv, wv_sb[:, k * 128:(k + 1) * 128],
                                         xt[(sb, k)][:], start=(k == 0),
                                         stop=(k == NKT - 1))
                # Q0 rope first (frees its PSUM slice earliest)
                q0s = tp.tile([128, QB], F32, tag="q0s", name=f"q0s{sb}")
                nc.scalar.copy(q0s[:], psq0)
                t, u = rope(f"q0_{sb}", q0s, s0)
                nc.vector.tensor_tensor(QT0[:, s0:s0 + QB], t[:], u[:], ADD)
                # K chain: SSM -> rope -> boundary gather
                hk = tp.tile([128, QB], F32, tag="hk", name=f"hk{sb}")
                nc.vector.tensor_tensor_scan(hk[:], fc[:, AKP:AKP + QB], psk,
                                             0.0, MUL, ADD)
                kp = tp.tile([128, QB], F32, tag="kp", name=f"kp{sb}")
                nc.vector.scalar_tensor_tensor(kp[:], hk[:], fc[:, CBK:CBK + 1],
                                               psk, MUL, ADD)
                t, u = rope(f"k_{sb}", kp, s0)
                nc.vector.tensor_tensor(KTx[:, 1 + s0:1 + s0 + QB], t[:], u[:],
                                        ADD)
                nc.gpsimd.tensor_copy(KCt[:, 4 + 64 * sb:4 + 64 * (sb + 1)],
                                      KTx[:, s0 + 8:s0 + 513:8])
                if sb == 0:
                    nc.gpsimd.tensor_copy(KCt[:, 0:4], KTx[:, 1:5])
                # Q1 rope
                q1s = tp.tile([128, QB], F32, tag="q1s", name=f"q1s{sb}")
                nc.scalar.copy(q1s[:], psq1)
                t, u = rope(f"q1_{sb}", q1s, s0)
                nc.vector.tensor_tensor(QT1[:, s0:s0 + QB], t[:], u[:], ADD)
                # V chain: SSM -> gather -> transpose
                hv = tp.tile([128, QB], F32, tag="hv", name=f"hv{sb}")
                nc.vector.tensor_tensor_scan(hv[:], fc[:, AVP:AVP + QB], psv,
                                             0.0, MUL, ADD)
                nc.vector.scalar_tensor_tensor(VTx[:, 1 + s0:1 + s0 + QB],
                                               hv[:], fc[:, CBV:CBV + 1], psv,
                                               MUL, ADD)
                nc.gpsimd.tensor_copy(VG[:, 4 + 64 * sb:4 + 64 * (sb + 1)],
                                      VTx[:, s0 + 8:s0 + 513:8])
                if sb == 0:
                    nc.gpsimd.tensor_copy(VG[:, 0:4], VTx[:, 1:5])

                dst, lo, n = [(vcG0, G0_LO, G0_N), (vcC1, C1_LO, C1_N),
                              (vcG1, G1_LO, G1_N), (vcG2, G2_LO, G2_N)][sb]
                pst = at.tile([n, 128], BF16, tag="st", bufs=2,
                              name=f"tr{sb}")
                nc.tensor.transpose(pst[:], VG[:, lo:lo + n],
                                    bc[:, IDENT:IDENT + 128])
                nc.scalar.copy(dst[:], pst[:])

            # per q-block key groups: (name, lo, n, mask (rows, col) or None)
            GROUPS = [
                [('G0', G0_LO, G0_N, (0, MG0B0))],
                [('G0', G0_LO, G0_N, None), ('C1', C1_LO, C1_N, (0, MG2B3))],
                [('G0', G0_LO, G0_N, None), ('G1', G1_LO, G1_N, (0, MG1B2))],
                [('G0', G0_LO, G0_N, None), ('G1', G1_LO, G1_N, None),
                 ('G2', G2_LO, G2_N, (0, MG2B3))],
            ]

            def attn_scores(b, h):
                q0 = QB * b
                QTh = QT0 if h == 0 else QT1
                plist = []
                for (gn, lo, n, mk) in GROUPS[b]:
                    st = at.tile([n, QB], F32, tag="st", bufs=2,
                                 name=f"st{b}{h}{gn}")
                    nc.tensor.matmul(st[:], KCt[:, lo:lo + n],
                                     QTh[:, q0:q0 + QB], start=True, stop=True)
                    if mk is not None:
                        r0, mc = mk
                        nc.vector.tensor_tensor(st[:], st[:],
                                                fc[r0:r0 + n, mc:mc + QB], ADD)
                    P = tp.tile([n, QB], BF16, tag="P", bufs=6,
                                name=f"P{b}{h}{gn}")
                    nc.scalar.activation(P[:], st[:], EXP, scale=SCALE)
                    plist.append((P, n, VCS[gn]))
                # t-1 diagonal: z = q . k_shifted, col-summed on PE
                z = tp.tile([128, QB], BF16, tag="z", name=f"z{b}{h}")
                nc.gpsimd.tensor_tensor(z[:], QTh[:, q0:q0 + QB],
                                        KTx[:, q0:q0 + QB], MUL)
                sd = at.tile([1, QB], F32, tag="sm", name=f"sd{b}{h}")
                nc.tensor.matmul(sd[:], bc[:, ONES:ONES + 1],
                                 z[:], start=True, stop=False)
                nc.tensor.matmul(sd[:], bc[0:1, ONES:ONES + 1],
                                 bc[0:1, MDIAG + QB * b:MDIAG + QB * (b + 1)],
                                 start=False, stop=True)
                pd = tp.tile([1, QB], BF16, tag="pd", bufs=4, name=f"pd{b}{h}")
                nc.scalar.activation(pd[:], sd[:], EXP, scale=SCALE)
                return plist, pd

            def attn_tail(b, h, plist, pd):
                q0 = QB * b
                OTh = OT0 if h == 0 else OT1
                dent = at.tile([1, QB], F32, tag="sm", name=f"den{b}{h}")
                den = dent[:]
                oun = at.tile([128, QB], F32, tag="oun", name=f"oun{b}{h}")
                npl = len(plist)
                for i, (P, n, _) in enumerate(plist):
                    nc.tensor.matmul(den, bc[0:n, ONES:ONES + 1], P[:],
                                     start=(i == 0), stop=False)
                nc.tensor.matmul(den, bc[0:1, ONES:ONES + 1], pd[:],
                                 start=False, stop=True)
                for i, (P, n, vt) in enumerate(plist):
                    nc.tensor.matmul(oun[:], vt[:], P[:],
                                     start=(i == 0), stop=(i == npl - 1))
                # diagonal V term: oun += bcast(pd) * V_shifted
                psb = at.tile([128, QB], F32, tag="st", bufs=2,
                              name=f"psb{b}{h}")
                nc.tensor.matmul(psb[:], bc[0:1, ONES:ONES + 128], pd[:],
                                 start=True, stop=True)
                vsh = tp.tile([128, QB], F32, tag="vsh", name=f"vsh{b}{h}")
                nc.vector.tensor_tensor(vsh[:], psb[:], VTx[:, q0:q0 + QB], MUL)
                nc.vector.tensor_tensor(oun[:], oun[:], vsh[:], ADD)
                # normalize
                rec = tp.tile([1, QB], F32, tag="rec", bufs=4,
                              name=f"rec{b}{h}")
                nc.vector.reciprocal_approx_fast(rec[:], den)
                recb = tp.tile([1, QB], BF16, tag="recb", bufs=4,
                               name=f"recb{b}{h}")
                nc.scalar.copy(recb[:], rec[:])
                rb = at.tile([128, QB], F32, tag="st", bufs=2, name=f"rb{b}{h}")
                nc.tensor.matmul(rb[:], bc[0:1, ONES:ONES + 128], recb[:],
                                 start=True, stop=True)
                rbs = tp.tile([128, QB], F32, tag="rbs", name=f"rbs{b}{h}")
                nc.scalar.copy(rbs[:], rb[:])
                nc.vector.tensor_tensor(OTh[:, q0:q0 + QB], oun[:], rbs[:], MUL)

            def attn_scores_pair(b):
                p0, d0 = attn_scores(b, 0)
                p1, d1 = attn_scores(b, 1)
                return p0, d0, p1, d1

            def attn_tail_pair(b, a):
                p0, d0, p1, d1 = a
                attn_tail(b, 0, p0, d0)
                attn_tail(b, 1, p1, d1)

            def wo_block(sb):
                s0 = QB * sb
                for g in range(8):           # dtile pairs {2g, 2g+1}
                    yp = pj.tile([128, 2 * QB], F32,
                                 tag="qq" if g % 2 == 0 else "kv",
                                 name=f"yp{sb}_{g}")
                    for j in range(2):
                        d0 = 128 * (2 * g + j)
                        nc.tensor.matmul(yp[:, QB * j:QB * (j + 1)],
                                         wo_sb[:, d0:d0 + 128],
                                         OT0[:, s0:s0 + QB], start=True,
                                         stop=False)
                        nc.tensor.matmul(yp[:, QB * j:QB * (j + 1)],
                                         wo_sb[:, D + d0:D + d0 + 128],
                                         OT1[:, s0:s0 + QB], start=False,
                                         stop=True)
                    yts = tp.tile([128, 2 * QB], BF16, tag="yts", bufs=4,
                                  name=f"yts{sb}_{g}")
                    if g % 4 == 3:
                        nc.vector.tensor_copy(yts[:], yp[:])
                    else:
                        nc.scalar.copy(yts[:], yp[:])
                    for j in range(2):
                        d0 = 128 * (2 * g + j)
                        nc.sync.dma_start(yT[d0:d0 + 128, s0:s0 + QB],
                                          yts[:, QB * j:QB * (j + 1)])

            # ---- schedule: keep PE busy, hide DVE/scalar latency ----
            proj_sblock(0)
            load_x(2)
            proj_sblock(1)
            load_x(3)
            a0 = attn_scores_pair(0)
            proj_sblock(2)
            attn_tail_pair(0, a0)
            a1 = attn_scores_pair(1)
            proj_sblock(3)
            attn_tail_pair(1, a1)
            a2 = attn_scores_pair(2)
            wo_block(0)
            attn_tail_pair(2, a2)
            a3 = attn_scores_pair(3)
            wo_block(1)
            attn_tail_pair(3, a3)
            wo_block(2)
            wo_block(3)
            for nm, t in [("dQT0", QT0), ("dKTx", KTx), ("dVTx", VTx),
                          ("dKCt", KCt), ("dVG", VG), ("dOT0", OT0),
                          ("dOT1", OT1)]:
                nc.sync.dma_start(dbg[nm][:, :], t[:])

    nc.compile()
    return nc


def _softplus(x):
    return np.log1p(np.exp(-np.abs(x))) + np.maximum(x, 0)


def _host_prep(inputs):
    import ml_dtypes
    bf = ml_dtypes.bfloat16
    x = np.asarray(inputs['x'], np.float32)
    freq = np.asarray(inputs['freq_cis'], np.float32)
    wq = np.asarray(inputs['wq'], np.float32)
    wk = np.asarray(inputs['wk'], np.float32)
    wv = np.asarray(inputs['wv'], np.float32)
    wo = np.asarray(inputs['wo'], np.float32)
    alk = np.asarray(inputs['a_log_k'], np.float32)
    bk = np.asarray(inputs['b_k'], np.float32)
    ck = np.asarray(inputs['c_k'], np.float32)
    alv = np.asarray(inputs['a_log_v'], np.float32)
    bv = np.asarray(inputs['b_v'], np.float32)
    cv = np.asarray(inputs['c_v'], np.float32)

    perm = np.concatenate([np.arange(0, HD, 2), np.arange(1, HD, 2)])
    xT = np.ascontiguousarray(x[0].T).astype(bf)            # (D, S)
    cos = np.ascontiguousarray(freq[:, :, 0, 0].T)          # (64, S)
    sin = np.ascontiguousarray(freq[:, :, 1, 0].T)
    cos2 = np.ascontiguousarray(np.concatenate([cos, cos], 0))
    sin2 = np.ascontiguousarray(np.concatenate([sin, -sin], 0))

    # ---- masks ----
    col = np.arange(QB)
    band = np.full((64, QB), NEG, np.float32)
    for r in range(64):
        band[r, 8 * r + 8:] = 0.0
    # G0 = [first4 | chunk0] for b=0: first4 key k valid iff k<=t;
    # chunk0 boundary banded.
    mG0b0 = np.full((128, QB), NEG, np.float32)
    for k in range(4):
        mG0b0[k, k:] = 0.0
    mG0b0[4:68] = band
    # G1 = [chunk1 | chunk2] for b=2: chunk1 all valid, chunk2 banded.
    mG1b2 = np.zeros((128, QB), np.float32)
    mG1b2[64:128] = band
    # G2 (b=3) and C1 (b=1): plain band in rows 0:64.
    mG2b3 = np.full((128, QB), NEG, np.float32)
    mG2b3[0:64] = band

    f32c = np.zeros((128, F32C_W), np.float32)
    f32c[:, MG0B0:MG0B0 + QB] = mG0b0
    f32c[:, MG1B2:MG1B2 + QB] = mG1b2
    f32c[:, MG2B3:MG2B3 + QB] = mG2b3
    f32c[0, ONESF:ONESF + 128] = 1.0

    t = np.arange(S)
    mdiag = np.where((t >= 5) & (t % 8 != 0), 0.0, NEG).astype(np.float32)
    mdiag = mdiag.reshape(4, QB)

    bf16c = np.zeros((128, BF16C_W), np.float32)
    bf16c[0, MDIAG:MDIAG + S] = mdiag.reshape(-1)
    bf16c[:, IDENT:IDENT + 128] = np.eye(128, dtype=np.float32)
    bf16c[:, ONES:ONES + 128] = 1.0

    ak_full = np.exp(-_softplus(alk.astype(np.float64))).astype(np.float32)
    av_full = np.exp(-_softplus(alv.astype(np.float64))).astype(np.float32)

    def pack_w(wslice):
        # (out_n, D) -> SBUF layout (128, NKT*out_n): [p, k*out_n + j]
        out_n = wslice.shape[0]
        wT = np.ascontiguousarray(wslice.T)  # (D, out_n)
        return np.ascontiguousarray(
            wT.reshape(NKT, 128, out_n).transpose(1, 0, 2).reshape(128, -1)
        ).astype(bf)

    shared = {
        "xT": xT, "cosk": cos2, "sink": sin2,
        "bf16c": bf16c.astype(bf),
    }

    in_maps = []
    for c in range(8):
        g = c // 2
        wq_c = wq[256 * c:256 * (c + 1)]
        wq_cp = np.concatenate(
            [wq_c[h * HD:(h + 1) * HD][perm] for h in range(2)])
        wk_g = wk[128 * g:128 * (g + 1)][perm]
        wv_g = wv[128 * g:128 * (g + 1)]
        ak = ak_full[128 * g:128 * (g + 1)][perm]
        bk_g = bk[128 * g:128 * (g + 1)][perm]
        ck_g = ck[128 * g:128 * (g + 1)][perm]
        av_ = av_full[128 * g:128 * (g + 1)]
        bv_g = bv[128 * g:128 * (g + 1)]
        cv_g = cv[128 * g:128 * (g + 1)]

        fcl = f32c.copy()
        fcl[:, AKP:AKP + QB] = np.where(col[None, :] % KC == 0, 0.0,
                                        ak[:, None])
        fcl[:, AVP:AVP + QB] = np.where(col[None, :] % KC == 0, 0.0,
                                        av_[:, None])
        fcl[:, CBK] = ck_g * bk_g
        fcl[:, CBV] = cv_g * bv_g

        wo_pack = np.concatenate(
            [np.ascontiguousarray(wo[:, 256 * c:256 * c + 128].T),
             np.ascontiguousarray(wo[:, 256 * c + 128:256 * (c + 1)].T)],
            axis=1).astype(bf)   # (128, 2*D)

        m = dict(shared)
        m.update({
            "wq": pack_w(wq_cp),
            "wk": pack_w(wk_g),
            "wv": pack_w(wv_g),
            "wo": wo_pack,
            "f32c": fcl,
        })
        in_maps.append(m)
    return in_maps


def kernel(**inputs) -> np.ndarray:
    if 'nc' not in _CACHE:
        _CACHE['nc'] = _build_module()
    nc = _CACHE['nc']
    in_maps = _host_prep(inputs)
    res = bass_utils.run_bass_kernel_spmd(nc, in_maps, core_ids=list(range(8)),
                                          **_CACHE.get('run_kwargs', {}))
    _CACHE['last_result'] = res
    yT = res.results[0]["yT"].astype(np.float64)
    for c in range(1, 8):
        yT += res.results[c]["yT"]
    return np.ascontiguousarray(yT.T[None]).astype(np.float32)
